# revision 6
# baseline (speedup 1.0000x reference)
"""AttentionalJoin kernel for 8 Trainium2 NeuronCores.

Math: the reference builds full (M x M) self-attention over M = N+1 tokens
(CLS prepended) but returns only the CLS row of the projected output.  Only
the CLS query survives, so attention collapses to a softmax-weighted token
pooling:

    q       = Wq @ cls                       (per head h: q_h)
    score_t = scale * q_h . (Wk x_t)_h  =  x_t . R[:, h],   R = scale*Wk_h^T q_h
    p       = softmax over the M tokens (scores bounded ~[-6, 6]; no max-sub)
    pooled_h = sum_t p_t x_t                 (linearity: project AFTER pooling)
    out     = proj( concat_h Wv_h pooled_h ) + proj_b

The device streams x once (memory-bound part): scores = X @ R, exp, and the
weighted token-sum + partition Z per head; x is streamed in fp16 (rel err
~3e-4, PSUM accumulation stays fp32).  X^T for the scores matmul comes from
the DMA xbar transpose (16-bit only), which keeps the PE array free for the
real matmuls.  The tiny tail (head-mix with Wv, proj, bias, cls-token
contribution — ~10 MFLOP on 256 KB) runs on host in fp32.

Sharding: data-parallel over the batch dim, 2 batches per core.
"""

import numpy as np

H = 8
C = 512
HD = C // H
B = 16
N = 2048
NCORES = 8
BPC = B // NCORES          # batches per core
TOK = BPC * N              # tokens per core (4096)
NCHUNK = TOK // 512        # 512-token compute chunks per core (8)
MAX_DRAIN_WAITS = 1        # this walrus rejects instructions w/ >1 sem wait

_cached = {}


def _patch_drain():
    """The container's walrus codegen rejects instructions carrying more
    than one sem wait ("Too many sync wait commands").  Split extra waits
    onto dedicated same-engine NOPs, which preserves semantics (engine
    queues are in-order)."""
    import concourse.tile as tile_mod
    from concourse import mybir
    from bass_rust import ScopedClock

    if getattr(tile_mod.TileContext, "_drain_patched", False):
        return

    orig_lower = tile_mod.TileContext._lower_ordered_insts

    def _lower_ordered_insts(self, ordered):
        nc = self.nc
        for bbname, insts in ordered.items():
            out = []
            for inst in insts:
                si = inst.sync_info
                if si is not None and si.on_wait and len(si.on_wait) > MAX_DRAIN_WAITS:
                    waits = list(si.on_wait)
                    extra, keep = waits[:-MAX_DRAIN_WAITS], waits[-MAX_DRAIN_WAITS:]
                    for w in extra:
                        nop = mybir.InstNoOp(
                            name=f"waitsplit-{nc.next_id()}",
                            engine=inst.engine,
                            ins=[],
                            outs=[],
                            bass_nofuse=True,
                            sync_info=mybir.SyncInfo(on_wait=[w], on_update=[]),
                            debug=inst.debug,
                        )
                        out.append(nop)
                    inst.sync_info = mybir.SyncInfo(
                        on_wait=keep, on_update=list(si.on_update)
                    )
                out.append(inst)
            ordered[bbname] = out
        return orig_lower(self, ordered)

    tile_mod.TileContext._lower_ordered_insts = _lower_ordered_insts

    def _drain_and_barrier(self, tick_clock, wait_clock):
        nc = self.nc
        probe = mybir.InstNoOp(
            name=f"drain-wait-probe-{nc.next_id()}",
            engine=mybir.EngineType.SP,
            ins=[],
            outs=[],
        )
        wait_clock.add_sem_waits(probe, ScopedClock({None: tick_clock.global_clock}))
        waits = list(probe.sync_info.on_wait) if probe.sync_info else []
        for i in range(0, len(waits), MAX_DRAIN_WAITS):
            chunk = waits[i : i + MAX_DRAIN_WAITS]
            nop = nc.sync.nop(nofuse=True, hint="drain_wait")
            nop.ins.sync_info = mybir.SyncInfo(on_wait=chunk, on_update=[])
        nc.sync.drain()

        nc.all_engine_barrier()
        popped = nc._tile_sem_poison_stack.pop()
        assert popped is self._sem_poison
        nc.clear_and_free_semaphores(list(self.sems.allocated().values()))
        nc.all_engine_barrier()

    tile_mod.TileContext._drain_and_barrier = _drain_and_barrier
    tile_mod.TileContext._drain_patched = True


def _build_module():
    import concourse.bass as bass
    import concourse.tile as tile
    from concourse import mybir
    from concourse.masks import make_identity

    _patch_drain()
    f16 = mybir.dt.float16
    f32 = mybir.dt.float32
    EXP = mybir.ActivationFunctionType.Exp

    nc = bass.Bass()
    x_in = nc.dram_tensor("x", [TOK, C], f16, kind="ExternalInput")
    r_in = nc.dram_tensor("r", [C, H], f16, kind="ExternalInput")
    s_out = nc.dram_tensor("s", [BPC, H, C], f32, kind="ExternalOutput")
    z_out = nc.dram_tensor("z", [BPC, H, N // 512], f32, kind="ExternalOutput")

    # natural-layout view: 4 chunks of 1024 tokens, 8 blocks of 128 each
    x_src = x_in.rearrange("(a j p) f -> a p j f", a=4, j=8, p=128)
    r_src = r_in.rearrange("(q p) h -> p q h", p=128)

    with tile.TileContext(nc) as tc:
        with (
            tc.tile_pool(name="xpool", bufs=1) as xpool,
            tc.tile_pool(name="consts", bufs=1) as consts,
            tc.tile_pool(name="xtpool", bufs=1) as xtpool,
            tc.tile_pool(name="epool", bufs=1) as epool,
            tc.tile_pool(name="opool", bufs=2) as opool,
            tc.tile_pool(name="psc", bufs=3, space="PSUM") as psc_pool,
            tc.tile_pool(name="pet", bufs=3, space="PSUM") as pet_pool,
            tc.tile_pool(name="ps", bufs=1, space="PSUM") as ps_pool,
        ):
            ident = consts.tile([128, 128], f16)
            make_identity(nc, ident)
            r_sb = consts.tile([128, 4, H], f16)
            nc.sync.dma_start(out=r_sb, in_=r_src)

            # x natural layout (rhs of the pooling matmul) via SWDGE so it
            # doesn't queue behind the xbar transposes on the HWDGE rings
            x_sb = []
            for A in range(4):
                t = xpool.tile([128, 8, C], f16, tag=f"x{A}", name=f"x{A}")
                nc.gpsimd.dma_start(out=t, in_=x_src[A])
                x_sb.append(t)

            # x^T via DMA xbar transpose straight from DRAM:
            # (1024 tokens, 128 cs) -> (128 cs, 1024 tokens), 16 transfers.
            # A-major so chunk 0's four c-chunks land first; alternate the
            # two HWDGE rings (SP + ACT) to halve per-ring serialization.
            xt = [
                xtpool.tile([128, TOK], f16, tag=f"xt{q}", name=f"xt{q}")
                for q in range(4)
            ]
            for A in range(4):
                for q in range(4):
                    eng = nc.sync if q % 2 == 0 else nc.scalar
                    eng.dma_start(
                        out=xt[q][:, A * 1024 : (A + 1) * 1024],
                        in_=x_in[A * 1024 : (A + 1) * 1024, q * 128 : (q + 1) * 128],
                        transpose=True,
                    )

            e_sb = [epool.tile([H, N], f16, tag=f"e{b}", name=f"e{b}") for b in range(BPC)]
            zp = [
                epool.tile([H, N // 512], f32, tag=f"zp{b}", name=f"zp{b}")
                for b in range(BPC)
            ]
            et = [
                epool.tile([128, 16, H], f16, tag=f"et{b}", name=f"et{b}")
                for b in range(BPC)
            ]
            ps = [ps_pool.tile([H, C], f32, tag=f"ps{b}", name=f"psacc{b}") for b in range(BPC)]

            ncopy = 0
            for a in range(NCHUNK):
                b, g = divmod(a, 4)
                psc = psc_pool.tile([H, 512], f32, tag="psc", name=f"psc{a}")
                for q in range(4):
                    nc.tensor.matmul(
                        psc,
                        r_sb[:, q, :],
                        xt[q][:, a * 512 : (a + 1) * 512],
                        start=(q == 0),
                        stop=(q == 3),
                    )
                nc.scalar.activation(
                    out=e_sb[b][:, g * 512 : (g + 1) * 512],
                    in_=psc,
                    func=EXP,
                    accum_out=zp[b][:, g : g + 1],
                )
                for jj in range(4):
                    j = g * 4 + jj
                    pet = pet_pool.tile([128, H], f32, tag="pet", name=f"pet{a}_{jj}")
                    # transpose E via a plain matmul: out = e_sliceT @ I8
                    nc.tensor.matmul(
                        pet,
                        e_sb[b][:, j * 128 : (j + 1) * 128],
                        ident[:H, :H],
                        start=True,
                        stop=True,
                    )
                    if ncopy % 2 == 0:
                        nc.vector.tensor_copy(et[b][:, j, :], pet)
                    else:
                        nc.scalar.copy(et[b][:, j, :], pet)
                    ncopy += 1
                for jj in range(4):
                    j = g * 4 + jj
                    nc.tensor.matmul(
                        ps[b],
                        et[b][:, j, :],
                        x_sb[a // 2][:, (a % 2) * 4 + jj, :],
                        start=(j == 0),
                        stop=(j == 15),
                    )

            for b in range(BPC):
                so = opool.tile([H, C], f32, tag=f"so{b}", name=f"so{b}")
                nc.vector.tensor_copy(so, ps[b])
                nc.gpsimd.dma_start(out=s_out[b], in_=so)
                nc.gpsimd.dma_start(out=z_out[b], in_=zp[b])

    return nc


def _get_module():
    if "nc" not in _cached:
        _cached["nc"] = _build_module()
    return _cached["nc"]


def _host_prep(cls, qkv_w):
    scale = HD ** -0.5
    c = cls.reshape(C).astype(np.float64)
    Wq = qkv_w[:C].astype(np.float64)
    Wk = qkv_w[C : 2 * C].astype(np.float64)
    q = Wq @ c
    qh = q.reshape(H, HD)
    Wkh = Wk.reshape(H, HD, C)
    R = (scale * np.einsum("hdc,hd->ch", Wkh, qh)).astype(np.float16)
    k0 = Wk @ c
    score0 = scale * np.einsum("hd,hd->h", qh, k0.reshape(H, HD))
    e0 = np.exp(score0)
    return R, e0


def kernel(x, cls, qkv_w, proj_w, proj_b):
    from concourse.bass_utils import run_bass_kernel_spmd

    x = np.asarray(x, dtype=np.float32)
    cls = np.asarray(cls, dtype=np.float32)
    qkv_w = np.asarray(qkv_w, dtype=np.float32)
    proj_w = np.asarray(proj_w, dtype=np.float32)
    proj_b = np.asarray(proj_b, dtype=np.float32)

    R, e0 = _host_prep(cls, qkv_w)
    Wv = qkv_w[2 * C :]

    x16 = np.ascontiguousarray(x.reshape(B * N, C).astype(np.float16))
    nc = _get_module()
    in_maps = [
        {"x": x16[i * TOK : (i + 1) * TOK], "r": R}
        for i in range(NCORES)
    ]
    res = run_bass_kernel_spmd(nc, in_maps, list(range(NCORES)))
    _cached["last_results"] = res

    s_dev = np.concatenate([res.results[i]["s"] for i in range(NCORES)], axis=0)
    z_dev = np.concatenate(
        [res.results[i]["z"].sum(axis=-1) for i in range(NCORES)], axis=0
    )

    # add the CLS token's own contribution, normalize, head-mix + proj
    cf = cls.reshape(C)
    s_full = s_dev + (e0[:, None] * cf[None, :]).astype(np.float32)[None]
    z_full = z_dev + e0.astype(np.float32)[None]
    v = s_full / z_full[:, :, None]
    o = np.einsum("hdc,bhc->bhd", Wv.reshape(H, HD, C), v).reshape(B, C)
    y = o @ proj_w.T + proj_b
    return y.astype(np.float32)


# revision 10
# speedup vs baseline: 1.2809x; 1.2809x over previous
"""AttentionalJoin kernel for 8 Trainium2 NeuronCores.

Math: the reference builds full (M x M) self-attention over M = N+1 tokens
(CLS prepended) but returns only the CLS row of the projected output.  Only
the CLS query survives, so attention collapses to a softmax-weighted token
pooling:

    q       = Wq @ cls                       (per head h: q_h)
    score_t = scale * q_h . (Wk x_t)_h  =  x_t . R[:, h],   R = scale*Wk_h^T q_h
    p       = softmax over the M tokens (scores bounded ~[-6, 6]; no max-sub)
    pooled_h = sum_t p_t x_t                 (linearity: project AFTER pooling)
    out     = proj( concat_h Wv_h pooled_h ) + proj_b

The device streams x once (memory-bound part): scores = X @ R, exp, and the
weighted token-sum + partition Z per head; x is streamed in fp16 (rel err
~3e-4, PSUM accumulation stays fp32).  X^T for the scores matmul comes from
the DMA xbar transpose (16-bit only), which keeps the PE array free for the
real matmuls.  The tiny tail (head-mix with Wv, proj, bias, cls-token
contribution — ~10 MFLOP on 256 KB) runs on host in fp32.

Sharding: data-parallel over the batch dim, 2 batches per core.
"""

import numpy as np

H = 8
C = 512
HD = C // H
B = 16
N = 2048
NCORES = 8
BPC = B // NCORES          # batches per core
TOK = BPC * N              # tokens per core (4096)
NCHUNK = TOK // 512        # 512-token compute chunks per core (8)
MAX_DRAIN_WAITS = 1        # this walrus rejects instructions w/ >1 sem wait

_cached = {}


def _patch_drain():
    """The container's walrus codegen rejects instructions carrying more
    than one sem wait ("Too many sync wait commands").  Split extra waits
    onto dedicated same-engine NOPs, which preserves semantics (engine
    queues are in-order)."""
    import concourse.tile as tile_mod
    from concourse import mybir
    from bass_rust import ScopedClock

    if getattr(tile_mod.TileContext, "_drain_patched", False):
        return

    orig_lower = tile_mod.TileContext._lower_ordered_insts

    def _lower_ordered_insts(self, ordered):
        nc = self.nc
        for bbname, insts in ordered.items():
            out = []
            for inst in insts:
                si = inst.sync_info
                if si is not None and si.on_wait and len(si.on_wait) > MAX_DRAIN_WAITS:
                    waits = list(si.on_wait)
                    extra, keep = waits[:-MAX_DRAIN_WAITS], waits[-MAX_DRAIN_WAITS:]
                    for w in extra:
                        nop = mybir.InstNoOp(
                            name=f"waitsplit-{nc.next_id()}",
                            engine=inst.engine,
                            ins=[],
                            outs=[],
                            bass_nofuse=True,
                            sync_info=mybir.SyncInfo(on_wait=[w], on_update=[]),
                            debug=inst.debug,
                        )
                        out.append(nop)
                    inst.sync_info = mybir.SyncInfo(
                        on_wait=keep, on_update=list(si.on_update)
                    )
                out.append(inst)
            ordered[bbname] = out
        return orig_lower(self, ordered)

    tile_mod.TileContext._lower_ordered_insts = _lower_ordered_insts

    def _drain_and_barrier(self, tick_clock, wait_clock):
        nc = self.nc
        probe = mybir.InstNoOp(
            name=f"drain-wait-probe-{nc.next_id()}",
            engine=mybir.EngineType.SP,
            ins=[],
            outs=[],
        )
        wait_clock.add_sem_waits(probe, ScopedClock({None: tick_clock.global_clock}))
        waits = list(probe.sync_info.on_wait) if probe.sync_info else []
        for i in range(0, len(waits), MAX_DRAIN_WAITS):
            chunk = waits[i : i + MAX_DRAIN_WAITS]
            nop = nc.sync.nop(nofuse=True, hint="drain_wait")
            nop.ins.sync_info = mybir.SyncInfo(on_wait=chunk, on_update=[])
        nc.sync.drain()

        nc.all_engine_barrier()
        popped = nc._tile_sem_poison_stack.pop()
        assert popped is self._sem_poison
        nc.clear_and_free_semaphores(list(self.sems.allocated().values()))
        nc.all_engine_barrier()

    tile_mod.TileContext._drain_and_barrier = _drain_and_barrier
    tile_mod.TileContext._drain_patched = True


def _build_module():
    import concourse.bass as bass
    import concourse.tile as tile
    from concourse import mybir
    from concourse.masks import make_identity

    _patch_drain()
    f16 = mybir.dt.float16
    f32 = mybir.dt.float32
    EXP = mybir.ActivationFunctionType.Exp

    nc = bass.Bass()
    x_in = nc.dram_tensor("x", [TOK, C], f16, kind="ExternalInput")
    r_in = nc.dram_tensor("r", [C, H], f16, kind="ExternalInput")
    s_out = nc.dram_tensor("s", [BPC, H, C], f32, kind="ExternalOutput")
    z_out = nc.dram_tensor("z", [BPC, H, N // 512], f32, kind="ExternalOutput")

    # natural-layout view: 4 chunks of 1024 tokens, 8 blocks of 128 each
    x_src = x_in.rearrange("(a j p) f -> a p j f", a=4, j=8, p=128)
    r_src = r_in.rearrange("(q p) h -> p q h", p=128)

    with tile.TileContext(nc) as tc:
        with (
            tc.tile_pool(name="xpool", bufs=1) as xpool,
            tc.tile_pool(name="consts", bufs=1) as consts,
            tc.tile_pool(name="xtpool", bufs=1) as xtpool,
            tc.tile_pool(name="epool", bufs=1) as epool,
            tc.tile_pool(name="opool", bufs=2) as opool,
            tc.tile_pool(name="pt", bufs=2, space="PSUM") as pt_pool,
            tc.tile_pool(name="psc", bufs=2, space="PSUM") as psc_pool,
            tc.tile_pool(name="pet", bufs=2, space="PSUM") as pet_pool,
            tc.tile_pool(name="ps", bufs=1, space="PSUM") as ps_pool,
        ):
            ident = consts.tile([128, 128], f16)
            make_identity(nc, ident)
            r_sb = consts.tile([128, 4, H], f16)
            nc.sync.dma_start(out=r_sb, in_=r_src)

            # x natural layout (rhs of the pooling matmul)
            x_sb = []
            for A in range(4):
                t = xpool.tile([128, 8, C], f16, tag=f"x{A}", name=f"x{A}")
                nc.sync.dma_start(out=t, in_=x_src[A])
                x_sb.append(t)

            # x^T built on the PE as plain matmuls against the identity:
            # out = x_blockT @ I  (fp16 weights -> fast weight load), then a
            # PSUM -> SBUF fp16 copy on DVE/ACT
            xt = [
                xtpool.tile([128, TOK], f16, tag=f"xt{q}", name=f"xt{q}")
                for q in range(4)
            ]

            e_sb = [epool.tile([H, N], f16, tag=f"e{b}", name=f"e{b}") for b in range(BPC)]
            zp = [
                epool.tile([H, N // 512], f32, tag=f"zp{b}", name=f"zp{b}")
                for b in range(BPC)
            ]
            et = [
                epool.tile([128, 16, H], f16, tag=f"et{b}", name=f"et{b}")
                for b in range(BPC)
            ]
            ps = [ps_pool.tile([H, C], f32, tag=f"ps{b}", name=f"psacc{b}") for b in range(BPC)]

            ncopy = 0

            def stage1(a):
                """x^T for chunk a: 16 identity matmuls + 4 PSUM->SBUF copies."""
                nonlocal ncopy
                A, half = divmod(a, 2)
                for q in range(4):
                    pt = pt_pool.tile([128, 512], f32, tag="pt", name=f"pt{a}_{q}")
                    for j in range(4):
                        nc.tensor.matmul(
                            pt[:, j * 128 : (j + 1) * 128],
                            x_sb[A][:, half * 4 + j, q * 128 : (q + 1) * 128],
                            ident,
                            start=True,
                            stop=True,
                        )
                    dst = xt[q][:, a * 512 : (a + 1) * 512]
                    if ncopy % 3 != 2:
                        nc.vector.tensor_copy(dst, pt)
                    else:
                        nc.scalar.copy(dst, pt)
                    ncopy += 1

            def stage2(a):
                nonlocal ncopy
                b, g = divmod(a, 4)
                psc = psc_pool.tile([H, 512], f32, tag="psc", name=f"psc{a}")
                for q in range(4):
                    nc.tensor.matmul(
                        psc,
                        r_sb[:, q, :],
                        xt[q][:, a * 512 : (a + 1) * 512],
                        start=(q == 0),
                        stop=(q == 3),
                    )
                nc.scalar.activation(
                    out=e_sb[b][:, g * 512 : (g + 1) * 512],
                    in_=psc,
                    func=EXP,
                    accum_out=zp[b][:, g : g + 1],
                )
                for jj in range(4):
                    j = g * 4 + jj
                    pet = pet_pool.tile([128, H], f32, tag="pet", name=f"pet{a}_{jj}")
                    # transpose E via a plain matmul: out = e_sliceT @ I8
                    nc.tensor.matmul(
                        pet,
                        e_sb[b][:, j * 128 : (j + 1) * 128],
                        ident[:H, :H],
                        start=True,
                        stop=True,
                    )
                    if ncopy % 2 == 0:
                        nc.vector.tensor_copy(et[b][:, j, :], pet)
                    else:
                        nc.scalar.copy(et[b][:, j, :], pet)
                    ncopy += 1
                for jj in range(4):
                    j = g * 4 + jj
                    nc.tensor.matmul(
                        ps[b],
                        et[b][:, j, :],
                        x_sb[a // 2][:, (a % 2) * 4 + jj, :],
                        start=(j == 0),
                        stop=(j == 15),
                    )

            # software pipeline: transpose chunk a while chunk a-1 computes
            stage1(0)
            for a in range(1, NCHUNK):
                stage1(a)
                stage2(a - 1)
            stage2(NCHUNK - 1)

            for b in range(BPC):
                so = opool.tile([H, C], f32, tag=f"so{b}", name=f"so{b}")
                nc.vector.tensor_copy(so, ps[b])
                nc.gpsimd.dma_start(out=s_out[b], in_=so)
                nc.gpsimd.dma_start(out=z_out[b], in_=zp[b])

    return nc


def _get_module():
    if "nc" not in _cached:
        _cached["nc"] = _build_module()
    return _cached["nc"]


def _host_prep(cls, qkv_w):
    scale = HD ** -0.5
    c = cls.reshape(C).astype(np.float64)
    Wq = qkv_w[:C].astype(np.float64)
    Wk = qkv_w[C : 2 * C].astype(np.float64)
    q = Wq @ c
    qh = q.reshape(H, HD)
    Wkh = Wk.reshape(H, HD, C)
    R = (scale * np.einsum("hdc,hd->ch", Wkh, qh)).astype(np.float16)
    k0 = Wk @ c
    score0 = scale * np.einsum("hd,hd->h", qh, k0.reshape(H, HD))
    e0 = np.exp(score0)
    return R, e0


def kernel(x, cls, qkv_w, proj_w, proj_b):
    from concourse.bass_utils import run_bass_kernel_spmd

    x = np.asarray(x, dtype=np.float32)
    cls = np.asarray(cls, dtype=np.float32)
    qkv_w = np.asarray(qkv_w, dtype=np.float32)
    proj_w = np.asarray(proj_w, dtype=np.float32)
    proj_b = np.asarray(proj_b, dtype=np.float32)

    R, e0 = _host_prep(cls, qkv_w)
    Wv = qkv_w[2 * C :]

    x16 = np.ascontiguousarray(x.reshape(B * N, C).astype(np.float16))
    nc = _get_module()
    in_maps = [
        {"x": x16[i * TOK : (i + 1) * TOK], "r": R}
        for i in range(NCORES)
    ]
    res = run_bass_kernel_spmd(nc, in_maps, list(range(NCORES)))
    _cached["last_results"] = res

    s_dev = np.concatenate([res.results[i]["s"] for i in range(NCORES)], axis=0)
    z_dev = np.concatenate(
        [res.results[i]["z"].sum(axis=-1) for i in range(NCORES)], axis=0
    )

    # add the CLS token's own contribution, normalize, head-mix + proj
    cf = cls.reshape(C)
    s_full = s_dev + (e0[:, None] * cf[None, :]).astype(np.float32)[None]
    z_full = z_dev + e0.astype(np.float32)[None]
    v = s_full / z_full[:, :, None]
    o = np.einsum("hdc,bhc->bhd", Wv.reshape(H, HD, C), v).reshape(B, C)
    y = o @ proj_w.T + proj_b
    return y.astype(np.float32)
